# revision 2
# baseline (speedup 1.0000x reference)
"""Trainium2 Bass kernel for nn_BinarizeLayer (chain Viterbi binarization).

Algorithm
---------
The reference is a 2-state Viterbi DP over an 8.4M-node chain.  Writing
d_i = a0_i - a1_i (score difference of the two states), the forward pass
collapses to the scalar recurrence

    d_i = e_i + clamp(d_{i-1}, -lam, lam),        e_i = 2*p_i - 1,

and the backtracking pass to

    label_{i-1} = label_i ? (d_{i-1} >= -lam) : (d_{i-1} > lam).

Conjugating the clamp recurrence by prefix sums (s = running sum of -e,
w = d - (-s) + lam) turns it into

    w_k = min(max(w_{k-1}, sbar_{k-1}), sbar_{k-1} + 2*lam),

which is exactly the hardware `tensor_tensor_scan` (op0=max, op1=min).
The backtracking pass is a reversed scan with op0=logical_and,
op1=logical_or over precomputed threshold bits
    gt_k = (d_k > lam)  <=>  w_k > sbar_k + 2*lam
    ge_k = (d_k >= -lam) <=> w_k >= sbar_k.

Sharding: the chain is split into 8 core slices x 128 partition rows of
8192 payload elements, each row padded with a 64-element halo on both
sides (clamp recurrences forget their initial state as soon as the walk
saturates a clamp bound, so a 64-step warm-up reproduces the exact
sequential fp32 state; validated against the sequential reference).
The global chain ends are padded with p=0.5 (e=0 exactly), which makes
the boundary conditions exact; the final-label boundary condition is
injected by overwriting gt/ge at the last halo column with (d > 0).
"""

import numpy as np

import concourse.bass as bass
import concourse.mybir as mybir
from concourse import tile
from concourse import bass_utils

LAM = 0.75
N = 8388608
NCORES = 8
P = 128          # partitions
W = 64           # halo / warm-up width
D = 8192         # payload elements per partition row
R = D + 2 * W    # row length incl. halos
ELL = 1024       # pipeline block width
NBLK = D // ELL


def _build():
    f32 = mybir.dt.float32
    i8 = mybir.dt.int8
    Alu = mybir.AluOpType
    Copy = mybir.ActivationFunctionType.Copy

    nc = bass.Bass()
    x = nc.dram_tensor("x", [P, R], f32, kind="ExternalInput")
    y = nc.dram_tensor("y", [P, D], i8, kind="ExternalOutput")

    with tile.TileContext(nc) as tc:
        with tc.tile_pool(name="big", bufs=1) as big:
            XT = big.tile([P, R], f32)        # input p, then ebar = 1-2p in place
            SB = big.tile([P, R + 1], f32)    # running sum of ebar; col0 = 0
            SP = big.tile([P, R + 1], f32)    # SB + 2*lam
            WT = XT                           # clamp walk overwrites consumed ebar
            GT = big.tile([P, R], f32)        # d > lam   (Pool int-op rule: all f32)
            GE = big.tile([P, R], f32)        # d >= -lam
            LB = big.tile([P, R], i8)         # labels
            TMP = big.tile([P, 1], f32)

            nc.vector.memset(SB[:, 0:1], 0.0)
            nc.vector.memset(SP[:, 0:1], 2 * LAM)

            blocks = [(b * ELL, ELL) for b in range(NBLK)] + [(D, 2 * W)]
            for (c0, bw) in blocks:
                nc.sync.dma_start(XT[:, c0:c0 + bw], x[:, c0:c0 + bw])
                # ebar = 1 - 2p (in place)
                nc.scalar.activation(XT[:, c0:c0 + bw], XT[:, c0:c0 + bw],
                                     Copy, bias=1.0, scale=-2.0)
                # chained running sum: SB[c+1] = SB[c] + ebar[c]
                init = 0.0 if c0 == 0 else SB[:, c0:c0 + 1]
                nc.vector.tensor_tensor_scan(
                    SB[:, c0 + 1:c0 + 1 + bw], XT[:, c0:c0 + bw],
                    XT[:, c0:c0 + bw], init, Alu.add, Alu.bypass)
                nc.scalar.activation(SP[:, c0 + 1:c0 + 1 + bw],
                                     SB[:, c0 + 1:c0 + 1 + bw],
                                     Copy, bias=2 * LAM)
                # chained clamp walk: w = min(max(w, SB_excl), SP_excl)
                winit = LAM if c0 == 0 else WT[:, c0 - 1:c0]
                nc.vector.tensor_tensor_scan(
                    WT[:, c0:c0 + bw], SB[:, c0:c0 + bw],
                    SP[:, c0:c0 + bw], winit, Alu.max, Alu.min)
                # threshold bits against inclusive sums
                nc.vector.tensor_tensor(GT[:, c0:c0 + bw], WT[:, c0:c0 + bw],
                                        SP[:, c0 + 1:c0 + 1 + bw], Alu.is_gt)
                nc.vector.tensor_tensor(GE[:, c0:c0 + bw], WT[:, c0:c0 + bw],
                                        SB[:, c0 + 1:c0 + 1 + bw], Alu.is_ge)

            # boundary sentinel at the last halo column: gt = ge = (d > 0)
            nc.scalar.activation(TMP[:], SB[:, R:R + 1], Copy, bias=LAM)
            nc.vector.tensor_tensor(GT[:, R - 1:R], WT[:, R - 1:R], TMP[:],
                                    Alu.is_gt)
            nc.vector.tensor_tensor(GE[:, R - 1:R], WT[:, R - 1:R], TMP[:],
                                    Alu.is_gt)

            # backtracking: reversed logical scan per block with W warm-up
            for s in range(NBLK):
                c0 = W + s * ELL
                wd = ELL + W
                nc.vector.tensor_tensor_scan(
                    LB[:, c0:c0 + wd][:, ::-1],
                    GE[:, c0:c0 + wd][:, ::-1],
                    GT[:, c0:c0 + wd][:, ::-1],
                    0.0, Alu.logical_and, Alu.logical_or)
                nc.sync.dma_start(y[:, c0 - W:c0 - W + ELL],
                                  LB[:, c0:c0 + ELL])
    return nc


def _legalize_waits(nc, limit=1):
    """Split instructions carrying more than `limit` sem-waits.

    This walrus build rejects instructions whose sync_info has more wait
    commands than the ISA encoding allows (Tile can accumulate several).
    Excess waits move onto NoOps prepended on the same engine, which
    preserves per-engine ordering semantics.
    """
    import concourse.mybir as mybir
    for fn in nc.m.functions:
        for blk in fn.blocks:
            insts = blk.instructions
            i = 0
            while i < len(insts):
                inst = insts[i]
                si = getattr(inst, "sync_info", None)
                if si is not None and si.on_wait and len(si.on_wait) > limit:
                    waits = list(si.on_wait)
                    inst.sync_info = mybir.SyncInfo(
                        on_wait=waits[-limit:], on_update=list(si.on_update))
                    pending = waits[:-limit]
                    for j in range(0, len(pending), limit):
                        nop = mybir.InstNoOp(
                            name=nc.get_next_instruction_name(),
                            sync_info=mybir.SyncInfo(
                                on_wait=pending[j:j + limit], on_update=[]),
                            bass_nofuse=True,
                            engine=inst.engine,
                        )
                        insts.insert(i, nop)
                        i += 1
                i += 1
    return nc


_nc_cache = None


def _get_nc():
    global _nc_cache
    if _nc_cache is None:
        _nc_cache = _legalize_waits(_build())
    return _nc_cache


def _in_maps(inputs: np.ndarray) -> list:
    p = np.ascontiguousarray(inputs, dtype=np.float32)
    assert p.shape == (N,)
    pad = np.full(W, 0.5, np.float32)
    pp = np.concatenate([pad, p, pad])
    nrows = N // D
    X = np.lib.stride_tricks.as_strided(pp, (nrows, R), (D * 4, 4))
    return [{"x": np.ascontiguousarray(X[k * P:(k + 1) * P])}
            for k in range(NCORES)]


def _gather(results) -> np.ndarray:
    lab = np.concatenate([np.asarray(results[k]["y"]).reshape(-1)
                          for k in range(NCORES)])
    return lab.astype(np.int32)


def kernel(inputs: np.ndarray) -> np.ndarray:
    res = bass_utils.run_bass_kernel_spmd(_get_nc(), _in_maps(inputs),
                                          core_ids=list(range(NCORES)))
    return _gather(res.results)

